# revision 33
# baseline (speedup 1.0000x reference)
"""Corner-pooling module kernel for 8 Trainium2 NeuronCores.

Reference computation (NCHW, fp32):
    p1 = relu(bn(conv3x3(x, w_p1)))          # 256 -> 128 ch
    p2 = relu(bn(conv3x3(x, w_p2)))          # 256 -> 128 ch
    cp1 = cummax(p1, axis=H, reverse=True)
    cp2 = cummax(p2, axis=W, reverse=True)
    r  = relu(bn(conv3x3(cp1+cp2, w_c1)) + bn(conv1x1(x, w_c2)))
    out = relu(bn(conv3x3(r, w_p3)))

Sharding: 8 cores = 4 samples x 2 H-halves.  BN folded into conv
weights host-side.

This version: 1D Winograd F(2,3) along W for every 3x3 conv, fp16
operands.  Each output-column PAIR costs 4 matmul columns per (ky,
input-slab) instead of 6, cutting PE work 1.5x; fp16 weights halve
LDWEIGHTS time (the co-bottleneck of the fp32r version).  Activations
live as even/odd column planes [P, rows, 66] (col 0/65 zero pads), so
the per-layer input transform (4 tensor ops/row-range, t0=O[c-1]-O[c],
t1=E+O, t2=O-E, t3=E[c]-E[c+1]) and the output combine (y0=m0+m1+m2,
y1=m1-m2-m3, via one ACT copy of m1 + 2 DVE + 2 DVE/GPSIMD tensor ops
+ 2 ACT relu's per chunk) are all unit-stride.  x's t-planes and E/O
planes are precomputed host-side.  Corner pooling in plane form:
H-cummax = two independent row chains (E,O); W-cummax = in-place
z=max(O,shift(E)), masked reverse flat scan, E=max(E,O).  The halo
exchange/colmax AllReduce machinery is the same as the fp32r version,
on 132-wide (E|O) rows.  Output is written fp16 and cast on host.
"""

import sys

sys.path.insert(0, "/opt/trn_rl_repo")

import numpy as np

import bass_rust
import concourse.bass as bass
import concourse.mybir as mybir
from concourse.bass_utils import run_bass_kernel_spmd
from concourse.tile import TileContext

F32 = mybir.dt.float32
F16 = mybir.dt.float16
EPS = 1e-5
AL = mybir.AluOpType
AF = mybir.ActivationFunctionType

B, CIN, H, W = 4, 256, 128, 128
P = 128
OH = 64             # own rows per core
C = 66              # plane cols: pad + 64 pairs + pad
HS = 66             # x/r plane rows: own 64 + 1 halo each side
SS = 68             # s plane rows: own 64 + 2 halo each side
NPC = 4
PCR = OH // NPC
NT = 512            # psum plane width / chunk size (flat pairs)
FLAT_P = OH * C     # 4224, p1/p2/p3 out flat size
FLAT_C = (OH + 2) * C  # 4356, c1 out flat size


def _chunks(total, nt):
    out = []
    q = 0
    while q < total:
        out.append((q, min(nt, total - q)))
        q += nt
    return out


def _row_chunks(row_list):
    # [(r0, nrows)] -> [(q, nt)] row-aligned
    return [(r0 * C, nr * C) for r0, nr in row_list]


def _fix_multiwaits(nc):
    """walrus in this container accepts at most ONE sem wait per
    instruction; split extras onto same-engine nops placed just before."""

    def steal_nop(eng):
        bi = nc.engines[eng].nop()
        ins = bi.ins
        cur = nc.cur_bb.bb
        lst = cur.instructions
        assert lst[-1] is ins or lst[-1].name == ins.name
        cur.instructions = lst[:-1]
        return ins

    for fn in nc.m.functions:
        for bb in fn.blocks:
            out = []
            changed = False
            for inst in bb.instructions:
                si = inst.sync_info
                waits = list(si.on_wait) if si is not None and si.on_wait else []
                if len(waits) > 1:
                    for wv in waits[:-1]:
                        nop = steal_nop(inst.engine)
                        nop.sync_info = bass_rust.SyncInfo(on_wait=[wv], on_update=[])
                        out.append(nop)
                    inst.sync_info = bass_rust.SyncInfo(
                        on_wait=[waits[-1]], on_update=list(si.on_update or [])
                    )
                    changed = True
                out.append(inst)
            if changed:
                bb.instructions = out
    return nc


def build_nc():
    nc = bass.Bass()

    xt = nc.dram_tensor("xt", [2, P, 4, HS, C], F16, kind="ExternalInput")
    xe = nc.dram_tensor("xe", [2, P, 2, HS, C], F16, kind="ExternalInput")
    gp1 = nc.dram_tensor("gp1", [P, 2, 3, 4, P], F16, kind="ExternalInput")
    gp2 = nc.dram_tensor("gp2", [P, 2, 3, 4, P], F16, kind="ExternalInput")
    gc1 = nc.dram_tensor("gc1", [P, 3, 4, 2, P], F16, kind="ExternalInput")
    gc2 = nc.dram_tensor("gc2", [P, 2, 2, P], F16, kind="ExternalInput")
    gp3 = nc.dram_tensor("gp3", [P, 2, 2, 3, 4, P], F16, kind="ExternalInput")
    bm = nc.dram_tensor("bm", [P, 12], F32, kind="ExternalInput")
    o = nc.dram_tensor("o", [2, P, OH, W], F16, kind="ExternalOutput")

    groups = [[0, 1], [2, 3], [4, 5], [6, 7]]

    with TileContext(nc) as tc:
        with (
            tc.tile_pool(name="const", bufs=1) as cpool,
            tc.tile_pool(name="wt", bufs=1) as wpool,
            tc.tile_pool(name="psum", bufs=2, space="PSUM") as psum,
            tc.tile_pool(name="epi", bufs=2) as epool,
            tc.tile_pool(name="scratch", bufs=1) as spool_s,
            tc.tile_pool(name="dram", bufs=1, space="DRAM") as dpool,
            tc.tile_pool(name="s", bufs=1) as spool,
            tc.tile_pool(name="x", bufs=1) as xpool,
        ):
            bmt = cpool.tile([P, 12], F32)
            nc.sync.dma_start(bmt[:], bm[:])
            m_top = bmt[:, 6:7]
            m_bot = bmt[:, 7:8]
            coloff = bmt[:, 8:9]

            # ---- weights: gp2 first (prewarm + early p2 chunks) ----
            gp1t = wpool.tile([P, 2, 3, 4, P], F16, tag="wg1")
            gp2t = wpool.tile([P, 2, 3, 4, P], F16, tag="wg2")
            nc.sync.dma_start(gp2t[:, 0], gp2[:, 0])
            nc.sync.dma_start(gp2t[:, 1], gp2[:, 1])
            gc1t = wpool.tile([P, 3, 4, 2, P], F16, tag="wgc1")
            gc2t = wpool.tile([P, 2, 2, P], F16, tag="wgc2")

            # ---- PE pre-warm on gp2 ----
            gp2fl = gp2t[:].rearrange("p a b c d -> p (a b c d)")
            ptw = psum.tile([P, 4, NT], F32, tag="ps")
            for _ in range(3):
                nc.tensor.matmul(
                    ptw[:, 0, :], gp2fl[:, 0:P], gp2fl[:, 0:NT],
                    start=True, stop=True,
                )

            # ---- x t-planes + E/O planes ----
            xtt = [xpool.tile([P, 4, HS, C], F16, name=f"xt{s}", tag=f"xt{s}")
                   for s in range(2)]
            xet = [xpool.tile([P, 2, HS, C], F16, name=f"xe{s}", tag=f"xe{s}")
                   for s in range(2)]
            starts = [60, 0, 6, 54, 48, 42, 36, 30, 24, 18, 12]
            for k, r0 in enumerate(starts):
                for s in range(2):
                    nc.sync.dma_start(
                        xtt[s][:, :, r0:r0 + 6, :], xt[s, :, :, r0:r0 + 6, :]
                    )
                if k == 1:
                    nc.sync.dma_start(gp1t[:], gp1[:])
                elif k == 2:
                    nc.sync.dma_start(gc2t[:], gc2[:])
                elif k == 3:
                    nc.sync.dma_start(gc1t[:], gc1[:])
                elif k == 5:
                    nc.sync.dma_start(xet[0][:], xe[0])
                elif k == 7:
                    nc.sync.dma_start(xet[1][:], xe[1])
            xtf = [[xtt[s][:, w4].rearrange("p h w -> p (h w)") for w4 in range(4)]
                   for s in range(2)]
            xef = [[xet[s][:, eo].rearrange("p h w -> p (h w)") for eo in range(2)]
                   for s in range(2)]

            # ---- s planes (p1 output rows 2..65; halo rows 0,1,66,67) ----
            sE = spool.tile([P, SS, C], F16, name="sE", tag="sE")
            sO = spool.tile([P, SS, C], F16, name="sO", tag="sO")
            sEf = sE[:].rearrange("p h w -> p (h w)")
            sOf = sO[:].rearrange("p h w -> p (h w)")
            # rows 66,67 zeroed so the colmax trees can use them as
            # identity padding (p1 output is post-relu, >= 0)
            for sp_ in (sE, sO):
                nc.gpsimd.memset(sp_[:, SS - 2:SS, :], 0.0)

            tcs = spool_s.tile([P, NT], F16, tag="tcs")

            def epilogue(pt, nt, bias, dstE, dstO, act_heavy=False):
                """y0 = relu(m0+m1+m2+b) -> dstE, y1 = relu(m1-m2-m3+b) -> dstO.
                act_heavy also copies m2 to SBUF via ACT so the tensor adds
                run on GPSIMD, freeing DVE (used under p1 where DVE owns the
                cummax chain).  GPSIMD cannot touch PSUM, so the PSUM-reading
                ops always stay on DVE."""
                ta = epool.tile([P, NT], F32, tag="ta")
                tb = epool.tile([P, NT], F32, tag="tb")
                nc.scalar.activation(ta[:, :nt], pt[:, 1, :nt], AF.Copy)
                if act_heavy:
                    nc.scalar.activation(tcs[:, :nt], pt[:, 2, :nt], AF.Copy)
                    nc.vector.tensor_tensor(tb[:, :nt], ta[:, :nt],
                                            pt[:, 0, :nt], AL.add)
                    nc.gpsimd.tensor_tensor(tb[:, :nt], tb[:, :nt],
                                            tcs[:, :nt], AL.add)
                    nc.scalar.activation(dstE, tb[:, :nt], AF.Relu, bias=bias)
                    nc.gpsimd.tensor_tensor(ta[:, :nt], ta[:, :nt],
                                            tcs[:, :nt], AL.subtract)
                    nc.vector.tensor_tensor(ta[:, :nt], ta[:, :nt],
                                            pt[:, 3, :nt], AL.subtract)
                    nc.scalar.activation(dstO, ta[:, :nt], AF.Relu, bias=bias)
                    return
                nc.vector.tensor_tensor(tb[:, :nt], ta[:, :nt], pt[:, 0, :nt], AL.add)
                nc.vector.tensor_tensor(tb[:, :nt], tb[:, :nt], pt[:, 2, :nt], AL.add)
                nc.scalar.activation(dstE, tb[:, :nt], AF.Relu, bias=bias)
                nc.vector.tensor_tensor(ta[:, :nt], ta[:, :nt], pt[:, 2, :nt],
                                        AL.subtract)
                nc.vector.tensor_tensor(ta[:, :nt], ta[:, :nt], pt[:, 3, :nt],
                                        AL.subtract)
                nc.scalar.activation(dstO, ta[:, :nt], AF.Relu, bias=bias)

            def conv_p(gt, bias, outEf, outOf, out_base, chunk_list,
                       post_chunk=None, act_heavy=False):
                """p1/p2-style conv: K=256 (2 slabs), 128 out ch."""
                for i, (q, nt) in enumerate(chunk_list):
                    pt = psum.tile([P, 4, NT], F32, tag="ps")
                    for w4 in range(4):
                        terms = [(gt[:, s, ky, w4, :], xtf[s][w4], ky * C)
                                 for s in range(2) for ky in range(3)]
                        for j, (lhsT, rf, off) in enumerate(terms):
                            nc.tensor.matmul(
                                pt[:, w4, :nt], lhsT, rf[:, q + off:q + off + nt],
                                start=(j == 0), stop=(j == len(terms) - 1),
                            )
                    epilogue(pt, nt, bias,
                             outEf[:, out_base + q:out_base + q + nt],
                             outOf[:, out_base + q:out_base + q + nt],
                             act_heavy=act_heavy)
                    if post_chunk is not None:
                        post_chunk(q, nt)

            # ---- conv p2 exchange chunks (rows 62-63 and 0-6) first ----
            with tc.tile_pool(name="p2", bufs=1) as p2pool:
                # 66 rows (only 0..63 used) so the slot fits r-half0 later
                p2E = p2pool.tile([P, HS, C], F16, name="p2E", tag="p2E")
                p2O = p2pool.tile([P, HS, C], F16, name="p2O", tag="p2O")
                p2Ef = p2E[:].rearrange("p h w -> p (h w)")
                p2Of = p2O[:].rearrange("p h w -> p (h w)")
                mk = p2pool.tile([P, PCR, C], F16)
                nc.vector.memset(mk[:], 1.0)
                nc.vector.memset(mk[:, :, 0:1], 0.0)
                nc.vector.memset(mk[:, :, C - 1:C], 0.0)
                mkf = mk[:].rearrange("p h w -> p (h w)")

                p2_rows = ([(62, 2), (0, 7)]
                           + [(r, min(7, 61 - r + 1)) for r in range(7, 62, 7)])
                p2_chunks = _row_chunks(p2_rows)
                conv_p(gp2t, bmt[:, 1:2], p2Ef, p2Of, 0, p2_chunks[:2])

                def wscan(rows_ap_E, rows_ap_O, mask_f):
                    # in-place W reverse cummax on an E/O row range
                    nc.vector.tensor_tensor(
                        rows_ap_O[:, :, 1:65], rows_ap_O[:, :, 1:65],
                        rows_ap_E[:, :, 2:66], AL.max,
                    )
                    flatO = rows_ap_O.rearrange("p h w -> p (h w)")
                    nc.vector.tensor_tensor_scan(
                        flatO[:, ::-1], flatO[:, ::-1], mask_f[:, ::-1],
                        0.0, AL.max, AL.mult,
                    )
                    nc.vector.tensor_tensor(
                        rows_ap_E[:], rows_ap_E[:], rows_ap_O[:], AL.max,
                    )

                for r in (62, 63, 0, 1):
                    nc.vector.memset(p2E[:, r:r + 1, C - 1:C], 0.0)
                    wscan(p2E[:, r:r + 1, :], p2O[:, r:r + 1, :],
                          mkf[:, 0:C])

                # ---- conv p1 (reverse order kept for DMA piece priority) ----
                p1_chunks = _chunks(FLAT_P, NT)
                conv_p(gp1t, bmt[:, 0:1], sEf, sOf, 2 * C,
                       list(reversed(p1_chunks)))

                # ---- colmax trees: the exchange payload rows are max-
                # reductions over the RAW p1 rows (max-reduce commutes with
                # the cummax chain), so the collective launches right after
                # conv p1 instead of after the 126-step chain.  Tree temps
                # live in p2 plane rows 21:53, which conv p2's later chunks
                # overwrite only after (WAR-ordered) the ct reads.
                def colmax_tree(sp, tdst, lo):
                    # tdst[0] = col-max of sp rows lo..lo+63 (rows 66,67 = 0)
                    nc.vector.tensor_tensor(
                        tdst[:, 0:32, :], sp[:, lo:lo + 32, :],
                        sp[:, lo + 32:lo + 64, :], AL.max)
                    n = 16
                    while n >= 1:
                        nc.vector.tensor_tensor(
                            tdst[:, 0:n, :], tdst[:, 0:n, :],
                            tdst[:, n:2 * n, :], AL.max)
                        n //= 2

                tE = p2E[:, 21:53, :]
                tO = p2O[:, 21:53, :]
                tE1 = p2E[:, 53:58, :]
                tO1 = p2O[:, 53:58, :]

                # ---- pairwise exchange (E|O concat, 132-wide rows).
                # ct build + u run on GPSIMD so they fire on data-readiness
                # instead of queueing behind the DVE backlog.
                ct = spool_s.tile([P, 8, 2 * C], F16, tag="exch")

                def ct_slot(k, se_, so_, m):
                    nc.gpsimd.tensor_scalar_mul(ct[:, k, 0:C], se_, m)
                    nc.gpsimd.tensor_scalar_mul(ct[:, k, C:2 * C], so_, m)

                colmax_tree(sE, tE, 2)       # tE[0] = cp1local row 2
                colmax_tree(sO, tO, 2)
                ct_slot(0, tE[:, 0, :], tO[:, 0, :], m_bot)
                colmax_tree(sE, tE, 3)       # tE[0] = cp1local row 3
                colmax_tree(sO, tO, 3)
                ct_slot(1, tE[:, 0, :], tO[:, 0, :], m_bot)
                # cp1local rows 64,65: max(raw64, raw65) and raw65 itself
                nc.vector.tensor_tensor(
                    tE1[:, 0, :], sE[:, 2 + 62, :], sE[:, 2 + 63, :], AL.max)
                nc.vector.tensor_tensor(
                    tO1[:, 0, :], sO[:, 2 + 62, :], sO[:, 2 + 63, :], AL.max)
                ct_slot(2, tE1[:, 0, :], tO1[:, 0, :], m_top)
                ct_slot(3, sE[:, 2 + OH - 1, :], sO[:, 2 + OH - 1, :], m_top)
                ct_slot(4, p2E[:, 0, :], p2O[:, 0, :], m_bot)
                ct_slot(5, p2E[:, 1, :], p2O[:, 1, :], m_bot)
                ct_slot(6, p2E[:, OH - 2, :], p2O[:, OH - 2, :], m_top)
                ct_slot(7, p2E[:, OH - 1, :], p2O[:, OH - 1, :], m_top)
                cc_in = dpool.tile([P, 8, 2 * C], F16)
                cc_out = dpool.tile([P, 8, 2 * C], F16)
                nc.sync.dma_start(cc_in[:], ct[:])
                nc.gpsimd.collective_compute(
                    "AllReduce", AL.add, replica_groups=groups,
                    ins=[cc_in[:]], outs=[cc_out[:]],
                )
                rx = spool_s.tile([P, 8, 2 * C], F16, tag="exch2")
                nc.sync.dma_start(rx[:], cc_out[:])

                # u = R[0] + coloff (top: partner colmax; bottom: -inf)
                u = spool_s.tile([P, 2 * C], F16, tag="u")
                nc.gpsimd.tensor_scalar_add(u[:], rx[:, 0, :], coloff)

                # ---- s-plane t-transform target ----
                with tc.tile_pool(name="st", bufs=1) as stpool:
                    st = stpool.tile([P, 4, SS, C], F16)
                    nc.vector.memset(st[:, :, :, 0:1], 0.0)
                    nc.vector.memset(st[:, :, :, C - 1:C], 0.0)
                    stf = [st[:, w4].rearrange("p h w -> p (h w)")
                           for w4 in range(4)]
                    cm = spool_s.tile([P, 2 * C], F16, tag="cm")
                    h0 = spool_s.tile([P, C], F16, tag="h0")
                    h1 = spool_s.tile([P, C], F16, tag="h1")

                    def st_xform(r0, nr):
                        # st rows r0..r0+nr from s rows (same tile rows)
                        args = [
                            (st[:, 0, r0:r0 + nr, 1:65],
                             sO[:, r0:r0 + nr, 0:64], sO[:, r0:r0 + nr, 1:65],
                             AL.subtract),
                            (st[:, 1, r0:r0 + nr, 1:65],
                             sE[:, r0:r0 + nr, 1:65], sO[:, r0:r0 + nr, 1:65],
                             AL.add),
                            (st[:, 2, r0:r0 + nr, 1:65],
                             sO[:, r0:r0 + nr, 1:65], sE[:, r0:r0 + nr, 1:65],
                             AL.subtract),
                            (st[:, 3, r0:r0 + nr, 1:65],
                             sE[:, r0:r0 + nr, 1:65], sE[:, r0:r0 + nr, 2:66],
                             AL.subtract),
                        ]
                        for i, (d, a, b_, op) in enumerate(args):
                            eng = nc.vector if i % 2 == 0 else nc.gpsimd
                            eng.tensor_tensor(d, a, b_, op)

                    def piece_scan(pc):
                        # u-independent part: W reverse cummax of the piece
                        # (max ops must stay on DVE; Pool has no max)
                        r0 = pc * PCR
                        nc.vector.memset(p2E[:, r0:r0 + PCR, C - 1:C], 0.0)
                        wscan(p2E[:, r0:r0 + PCR, :], p2O[:, r0:r0 + PCR, :],
                              mkf)

                    def piece_fix(pc):
                        # collective-dependent part: colmax fixup, s=cp1+cp2,
                        # pads, t-transform
                        r0 = pc * PCR
                        sr0 = 2 + r0
                        for sp_, uc in ((sE, 0), (sO, C)):
                            nc.vector.tensor_tensor(
                                sp_[:, sr0:sr0 + PCR, :],
                                sp_[:, sr0:sr0 + PCR, :],
                                u[:, None, uc:uc + C].to_broadcast((P, PCR, C)),
                                AL.max,
                            )
                        if pc == 0:
                            nc.vector.tensor_copy(cm[:, 0:C], sE[:, 2, :])
                            nc.vector.tensor_copy(cm[:, C:2 * C], sO[:, 2, :])
                        nc.gpsimd.tensor_tensor(
                            sE[:, sr0:sr0 + PCR, :], sE[:, sr0:sr0 + PCR, :],
                            p2E[:, r0:r0 + PCR, :], AL.add)
                        nc.gpsimd.tensor_tensor(
                            sO[:, sr0:sr0 + PCR, :], sO[:, sr0:sr0 + PCR, :],
                            p2O[:, r0:r0 + PCR, :], AL.add)
                        for sp_ in (sE, sO):
                            nc.gpsimd.memset(sp_[:, sr0:sr0 + PCR, 0:1], 0.0)
                            nc.gpsimd.memset(sp_[:, sr0:sr0 + PCR, C - 1:C], 0.0)
                        if pc == 0:
                            # halo rows: above (bottom cores) s rows 0,1
                            for j in range(2):
                                for sp_, cc0 in ((sE, 0), (sO, C)):
                                    nc.vector.tensor_tensor(
                                        h0[:], rx[:, 2 + j, cc0:cc0 + C],
                                        cm[:, cc0:cc0 + C], AL.max)
                                    nc.vector.tensor_tensor(
                                        h0[:], h0[:], rx[:, 6 + j, cc0:cc0 + C],
                                        AL.add)
                                    nc.vector.tensor_scalar_mul(
                                        sp_[:, j, :], h0[:], m_bot)
                            # below (top cores) s rows 66,67
                            for j in range(2):
                                for sp_, cc0 in ((sE, 0), (sO, C)):
                                    nc.vector.tensor_tensor(
                                        h1[:], rx[:, 0 + j, cc0:cc0 + C],
                                        rx[:, 4 + j, cc0:cc0 + C], AL.add)
                                    nc.vector.tensor_scalar_mul(
                                        sp_[:, SS - 2 + j, :], h1[:], m_top)
                            for sp_ in (sE, sO):
                                for rr in (0, SS - 2):
                                    nc.vector.memset(
                                        sp_[:, rr:rr + 2, 0:1], 0.0)
                                    nc.vector.memset(
                                        sp_[:, rr:rr + 2, C - 1:C], 0.0)
                            st_xform(0, 2)
                            st_xform(SS - 2, 2)
                        st_xform(sr0, PCR)

                    # ---- conv p2 remaining chunks.  The cummax chain (126
                    # DVE steps) and the u-independent W-scans drain under
                    # p2's matmuls; the u-dependent fixups run after, right
                    # as the collective result lands. ----
                    next_pc = [0]
                    chain_h = [OH]

                    def p2_post(q, nt):
                        hi = chain_h[0]
                        for h in range(hi, max(hi - 20, 1), -1):
                            nc.vector.tensor_tensor(
                                sE[:, h, :], sE[:, h, :], sE[:, h + 1, :],
                                AL.max)
                            nc.vector.tensor_tensor(
                                sO[:, h, :], sO[:, h, :], sO[:, h + 1, :],
                                AL.max)
                        chain_h[0] = max(hi - 20, 1)
                        covered = (q + nt) // C
                        while (next_pc[0] < NPC
                               and covered >= PCR * (next_pc[0] + 1)):
                            piece_scan(next_pc[0])
                            next_pc[0] += 1

                    conv_p(gp2t, bmt[:, 1:2], p2Ef, p2Of, 0, p2_chunks[2:],
                           post_chunk=p2_post)
                    for h in range(chain_h[0], 1, -1):
                        nc.vector.tensor_tensor(
                            sE[:, h, :], sE[:, h, :], sE[:, h + 1, :], AL.max)
                        nc.vector.tensor_tensor(
                            sO[:, h, :], sO[:, h, :], sO[:, h + 1, :], AL.max)
                    while next_pc[0] < NPC:
                        piece_scan(next_pc[0])
                        next_pc[0] += 1
                    for pc in range(NPC):
                        piece_fix(pc)

                    # gp3 into the slots gp1/gp2 free after their last chunks
                    # (emitted only now that every gp1/gp2 reader exists)
                    gp3t = [wpool.tile([P, 2, 3, 4, P], F16, name=f"gp3{t}",
                                       tag=t) for t in ("wg1", "wg2")]
                    nc.sync.dma_start(gp3t[0][:], gp3[:, 0])
                    nc.sync.dma_start(gp3t[1][:], gp3[:, 1])

                    # ---- r t-plane targets (xt slots; pads cleared) ----
                    rtt = []
                    for half in range(2):
                        rt_ = xpool.tile([P, 4, HS, C], F16, tag=f"xt{half}")
                        nc.vector.memset(rt_[:, :, :, 0:1], 0.0)
                        nc.vector.memset(rt_[:, :, :, C - 1:C], 0.0)
                        rtt.append(rt_)
                    rtf = [[rtt[s][:, w4].rearrange("p h w -> p (h w)")
                            for w4 in range(4)] for s in range(2)]

                    # ---- conv c1 (+ folded c2) -> r planes ----
                    c1_chunks = _chunks(FLAT_C, NT)
                    for half in range(2):
                        if half == 0:
                            rE = p2pool.tile([P, HS, C], F16, tag="p2E")
                            rO = p2pool.tile([P, HS, C], F16, tag="p2O")
                        else:
                            rE = spool.tile([P, HS, C], F16, tag="sE")
                            rO = spool.tile([P, HS, C], F16, tag="sO")
                        rEf = rE[:].rearrange("p h w -> p (h w)")
                        rOf = rO[:].rearrange("p h w -> p (h w)")
                        for i, (q, nt) in enumerate(c1_chunks):
                            pt = psum.tile([P, 4, NT], F32, tag="ps")
                            for w4 in range(4):
                                terms = [(gc1t[:, ky, w4, half, :], stf[w4],
                                          ky * C) for ky in range(3)]
                                if w4 == 0:
                                    terms += [(gc2t[:, s, half, :], xef[s][0], 0)
                                              for s in range(2)]
                                elif w4 == 3:
                                    terms += [(gc2t[:, s, half, :], xef[s][1], 0)
                                              for s in range(2)]
                                for j, (lhsT, rf, off) in enumerate(terms):
                                    nc.tensor.matmul(
                                        pt[:, w4, :nt], lhsT,
                                        rf[:, q + off:q + off + nt],
                                        start=(j == 0),
                                        stop=(j == len(terms) - 1),
                                    )
                            epilogue(pt, nt, bmt[:, 2 + half:3 + half],
                                     rEf[:, q:q + nt], rOf[:, q:q + nt],
                                     act_heavy=False)
                        # mask invalid halo rows, zero pads, transform to
                        # rt right away (overlaps the other half's matmuls)
                        for rp_ in (rE, rO):
                            nc.vector.tensor_scalar_mul(
                                rp_[:, 0, :], rp_[:, 0, :], m_bot)
                            nc.vector.tensor_scalar_mul(
                                rp_[:, HS - 1, :], rp_[:, HS - 1, :], m_top)
                            nc.vector.memset(rp_[:, :, 0:1], 0.0)
                            nc.vector.memset(rp_[:, :, C - 1:C], 0.0)
                        rt_ = rtt[half]
                        for r0, nr in ((0, 17), (17, 17), (34, 16), (50, 16)):
                            args = [
                                (rt_[:, 0, r0:r0 + nr, 1:65],
                                 rO[:, r0:r0 + nr, 0:64],
                                 rO[:, r0:r0 + nr, 1:65], AL.subtract),
                                (rt_[:, 1, r0:r0 + nr, 1:65],
                                 rE[:, r0:r0 + nr, 1:65],
                                 rO[:, r0:r0 + nr, 1:65], AL.add),
                                (rt_[:, 2, r0:r0 + nr, 1:65],
                                 rO[:, r0:r0 + nr, 1:65],
                                 rE[:, r0:r0 + nr, 1:65], AL.subtract),
                                (rt_[:, 3, r0:r0 + nr, 1:65],
                                 rE[:, r0:r0 + nr, 1:65],
                                 rE[:, r0:r0 + nr, 2:66], AL.subtract),
                            ]
                            for i, (d, a, b_, op) in enumerate(args):
                                eng = nc.vector if i % 2 == 0 else nc.gpsimd
                                eng.tensor_tensor(d, a, b_, op)

                    # ---- conv p3 -> interleaved fp16 staging -> out ----
                    p3_rows = [(r, 6) for r in range(0, 60, 6)] + [(60, 4)]
                    for half in range(2):
                        stg = xpool.tile([P, OH, W], F16, tag=f"xe{half}")
                        for i, (r0, nr) in enumerate(p3_rows):
                            q, nt = r0 * C, nr * C
                            pt = psum.tile([P, 4, NT], F32, tag="ps")
                            for w4 in range(4):
                                terms = [(gp3t[s][:, half, ky, w4, :],
                                          rtf[s][w4], ky * C)
                                         for s in range(2) for ky in range(3)]
                                for j, (lhsT, rf, off) in enumerate(terms):
                                    nc.tensor.matmul(
                                        pt[:, w4, :nt], lhsT,
                                        rf[:, q + off:q + off + nt],
                                        start=(j == 0),
                                        stop=(j == len(terms) - 1),
                                    )
                            ta = epool.tile([P, 6, C], F32, tag="ta")
                            tb = epool.tile([P, 6, C], F32, tag="tb")
                            taf = ta[:].rearrange("p h w -> p (h w)")
                            tbf = tb[:].rearrange("p h w -> p (h w)")
                            bias = bmt[:, 4 + half:5 + half]
                            nc.scalar.activation(taf[:, :nt], pt[:, 1, :nt],
                                                 AF.Copy)
                            nc.vector.tensor_tensor(
                                tbf[:, :nt], taf[:, :nt], pt[:, 0, :nt], AL.add)
                            nc.vector.tensor_tensor(
                                tbf[:, :nt], tbf[:, :nt], pt[:, 2, :nt], AL.add)
                            nc.scalar.activation(
                                stg[:, r0:r0 + nr, 0:W:2],
                                tb[:, :nr, 1:65], AF.Relu, bias=bias)
                            nc.vector.tensor_tensor(
                                taf[:, :nt], taf[:, :nt], pt[:, 2, :nt],
                                AL.subtract)
                            nc.vector.tensor_tensor(
                                taf[:, :nt], taf[:, :nt], pt[:, 3, :nt],
                                AL.subtract)
                            nc.scalar.activation(
                                stg[:, r0:r0 + nr, 1:W:2],
                                ta[:, :nr, 1:65], AF.Relu, bias=bias)
                            # per-chunk output DMA: rows ship the moment
                            # their relu lands, shrinking the kernel tail
                            nc.sync.dma_start(
                                o[half, :, r0:r0 + nr, :],
                                stg[:, r0:r0 + nr, :])

    _fix_multiwaits(nc)
    return nc


_NC = None


def _get_nc():
    global _NC
    if _NC is None:
        _NC = build_nc()
    return _NC


def _fold_bn(w, g, b, m, v):
    s = (np.asarray(g) / np.sqrt(np.asarray(v) + EPS)).astype(np.float32)
    t = (np.asarray(b) - np.asarray(m) * s).astype(np.float32)
    return np.asarray(w, np.float32) * s[:, None, None, None], t


def _wino_w(w):
    # w [O, I, 3, 3] -> G [4, 3ky, I, O]
    g0, g1, g2 = w[..., 0], w[..., 1], w[..., 2]
    G = np.stack([g0, (g0 + g1 + g2) * 0.5, (g0 - g1 + g2) * 0.5, g2])
    return G.transpose(0, 3, 2, 1).astype(np.float16)


def kernel(**inputs):
    x = np.asarray(inputs["x"], np.float32)

    w_p1, t_p1 = _fold_bn(inputs["w_p1"], inputs["g_p1"], inputs["b_p1"],
                          inputs["m_p1"], inputs["v_p1"])
    w_p2, t_p2 = _fold_bn(inputs["w_p2"], inputs["g_p2"], inputs["b_p2"],
                          inputs["m_p2"], inputs["v_p2"])
    w_c1, t_c1 = _fold_bn(inputs["w_c1"], inputs["g_c1"], inputs["b_c1"],
                          inputs["m_c1"], inputs["v_c1"])
    w_c2, t_c2 = _fold_bn(inputs["w_c2"], inputs["g_c2"], inputs["b_c2"],
                          inputs["m_c2"], inputs["v_c2"])
    w_p3, t_p3 = _fold_bn(inputs["w_p3"], inputs["g_p3"], inputs["b_p3"],
                          inputs["m_p3"], inputs["v_p3"])

    Gp1 = _wino_w(w_p1)  # [4,3,256,128]
    Gp2 = _wino_w(w_p2)
    Gc1 = _wino_w(w_c1)  # [4,3,128,256]
    Gp3 = _wino_w(w_p3)  # [4,3,256,256]

    gp1a = np.ascontiguousarray(
        Gp1.reshape(4, 3, 2, P, P).transpose(3, 2, 1, 0, 4))
    gp2a = np.ascontiguousarray(
        Gp2.reshape(4, 3, 2, P, P).transpose(3, 2, 1, 0, 4))
    gc1a = np.ascontiguousarray(
        Gc1.reshape(4, 3, P, 2, P).transpose(2, 1, 0, 3, 4))
    gp3a = np.ascontiguousarray(
        Gp3.reshape(4, 3, 2, P, 2, P).transpose(3, 2, 4, 1, 0, 5))
    gc2a = np.ascontiguousarray(
        w_c2[:, :, 0, 0].reshape(2, P, 2, P).transpose(3, 2, 0, 1)
    ).astype(np.float16)

    bias = np.zeros((P, 6), np.float32)
    bias[:, 0] = t_p1
    bias[:, 1] = t_p2
    bc = t_c1 + t_c2
    bias[:, 2] = bc[:P]
    bias[:, 3] = bc[P:]
    bias[:, 4] = t_p3[:P]
    bias[:, 5] = t_p3[P:]

    # x slabs per core-half with H halo, as fp16 E/O planes + t-planes
    x16 = x.astype(np.float16).astype(np.float32)
    xr = x16.reshape(B, 2, P, H, W)
    pad = np.zeros((B, 2, 2, P, HS, W), np.float32)  # [b, half, slab, p, h, w]
    pad[:, 0, :, :, 1:HS, :] = xr[:, :, :, 0:65, :]
    pad[:, 1, :, :, 0:HS - 1, :] = xr[:, :, :, 63:128, :]
    xE = np.zeros((B, 2, 2, P, HS, C), np.float32)
    xO = np.zeros_like(xE)
    xE[..., 1:65] = pad[..., 0::2]
    xO[..., 1:65] = pad[..., 1::2]
    t4 = np.zeros((B, 2, 2, P, 4, HS, C), np.float32)
    t4[..., 0, :, 1:65] = xO[..., 0:64] - xO[..., 1:65]
    t4[..., 1, :, 1:65] = xE[..., 1:65] + xO[..., 1:65]
    t4[..., 2, :, 1:65] = xO[..., 1:65] - xE[..., 1:65]
    t4[..., 3, :, 1:65] = xE[..., 1:65] - xE[..., 2:66]
    t4 = t4.astype(np.float16)
    xeo = np.stack([xE, -xO], axis=4).astype(np.float16)  # [b,half,slab,p,2,h,c]

    wmaps = {"gp1": gp1a, "gp2": gp2a, "gc1": gc1a, "gc2": gc2a, "gp3": gp3a}
    in_maps = []
    for b in range(B):
        for half in range(2):
            bmv = np.zeros((P, 12), np.float32)
            bmv[:, 0:6] = bias
            if half == 0:  # top
                bmv[:, 6] = 1.0
                bmv[:, 8] = 0.0
            else:  # bottom
                bmv[:, 7] = 1.0
                bmv[:, 8] = -1e30
            in_maps.append({
                "xt": t4[b, half], "xe": xeo[b, half], "bm": bmv, **wmaps,
            })

    global _last_in_maps
    _last_in_maps = in_maps

    nc = _get_nc()
    res = run_bass_kernel_spmd(nc, in_maps, list(range(8)))

    out = np.empty((B, CIN, H, W), np.float32)
    for b in range(B):
        out[b, :, 0:OH] = res.results[2 * b]["o"].reshape(CIN, OH, W)
        out[b, :, OH:H] = res.results[2 * b + 1]["o"].reshape(CIN, OH, W)
    return out


if __name__ == "__main__":
    import reference

    inp = {k: np.asarray(v) for k, v in reference.setup_inputs().items()}
    exp = np.asarray(reference.reference(**inp))
    got = kernel(**inp)
    err = np.abs(got - exp)
    rel = err.max() / max(np.abs(exp).max(), 1e-6)
    print("abs err max:", err.max(), "rel (vs absmax):", rel)


# revision 40
# speedup vs baseline: 1.0160x; 1.0160x over previous
"""Corner-pooling module kernel for 8 Trainium2 NeuronCores.

Reference computation (NCHW, fp32):
    p1 = relu(bn(conv3x3(x, w_p1)))          # 256 -> 128 ch
    p2 = relu(bn(conv3x3(x, w_p2)))          # 256 -> 128 ch
    cp1 = cummax(p1, axis=H, reverse=True)
    cp2 = cummax(p2, axis=W, reverse=True)
    r  = relu(bn(conv3x3(cp1+cp2, w_c1)) + bn(conv1x1(x, w_c2)))
    out = relu(bn(conv3x3(r, w_p3)))

Sharding: 8 cores = 4 samples x 2 H-halves.  BN folded into conv
weights host-side.

This version: 1D Winograd F(2,3) along W for every 3x3 conv, fp16
operands.  Each output-column PAIR costs 4 matmul columns per (ky,
input-slab) instead of 6, cutting PE work 1.5x; fp16 weights halve
LDWEIGHTS time (the co-bottleneck of the fp32r version).  Activations
live as even/odd column planes [P, rows, 66] (col 0/65 zero pads), so
the per-layer input transform (4 tensor ops/row-range, t0=O[c-1]-O[c],
t1=E+O, t2=O-E, t3=E[c]-E[c+1]) and the output combine (y0=m0+m1+m2,
y1=m1-m2-m3, via one ACT copy of m1 + 2 DVE + 2 DVE/GPSIMD tensor ops
+ 2 ACT relu's per chunk) are all unit-stride.  x's t-planes and E/O
planes are precomputed host-side.  Corner pooling in plane form:
H-cummax = two independent row chains (E,O); W-cummax = in-place
z=max(O,shift(E)), masked reverse flat scan, E=max(E,O).  The halo
exchange/colmax AllReduce machinery is the same as the fp32r version,
on 132-wide (E|O) rows.  Output is written fp16 and cast on host.
"""

import sys

sys.path.insert(0, "/opt/trn_rl_repo")

import numpy as np

import bass_rust
import concourse.bass as bass
import concourse.mybir as mybir
from concourse.bass_utils import run_bass_kernel_spmd
from concourse.tile import TileContext

F32 = mybir.dt.float32
F16 = mybir.dt.float16
EPS = 1e-5
AL = mybir.AluOpType
AF = mybir.ActivationFunctionType

B, CIN, H, W = 4, 256, 128, 128
P = 128
OH = 64             # own rows per core
C = 66              # plane cols: pad + 64 pairs + pad
HS = 66             # x/r plane rows: own 64 + 1 halo each side
SS = 68             # s plane rows: own 64 + 2 halo each side
NPC = 4
PCR = OH // NPC
NT = 256            # psum plane width / chunk size (flat pairs)
FLAT_P = OH * C     # 4224, p1/p2/p3 out flat size
FLAT_C = (OH + 2) * C  # 4356, c1 out flat size


def _chunks(total, nt):
    out = []
    q = 0
    while q < total:
        out.append((q, min(nt, total - q)))
        q += nt
    if len(out) >= 2 and out[-1][1] < 64:
        # balance a tiny tail across the last two chunks (even sizes)
        q0 = out[-2][0]
        rem = total - q0
        a = (rem // 2) & ~1
        out[-2] = (q0, a)
        out[-1] = (q0 + a, rem - a)
    return out


def _fix_multiwaits(nc):
    """walrus in this container accepts at most ONE sem wait per
    instruction; split extras onto same-engine nops placed just before."""

    def steal_nop(eng):
        bi = nc.engines[eng].nop()
        ins = bi.ins
        cur = nc.cur_bb.bb
        lst = cur.instructions
        assert lst[-1] is ins or lst[-1].name == ins.name
        cur.instructions = lst[:-1]
        return ins

    for fn in nc.m.functions:
        for bb in fn.blocks:
            out = []
            changed = False
            for inst in bb.instructions:
                si = inst.sync_info
                waits = list(si.on_wait) if si is not None and si.on_wait else []
                if len(waits) > 1:
                    for wv in waits[:-1]:
                        nop = steal_nop(inst.engine)
                        nop.sync_info = bass_rust.SyncInfo(on_wait=[wv], on_update=[])
                        out.append(nop)
                    inst.sync_info = bass_rust.SyncInfo(
                        on_wait=[waits[-1]], on_update=list(si.on_update or [])
                    )
                    changed = True
                out.append(inst)
            if changed:
                bb.instructions = out
    return nc


def build_nc():
    nc = bass.Bass()

    xt = nc.dram_tensor("xt", [2, P, 4, HS, C], F16, kind="ExternalInput")
    xe = nc.dram_tensor("xe", [2, P, 2, HS, C], F16, kind="ExternalInput")
    gp1 = nc.dram_tensor("gp1", [P, 2, 3, 4, P], F16, kind="ExternalInput")
    gp2 = nc.dram_tensor("gp2", [P, 2, 3, 4, P], F16, kind="ExternalInput")
    gc1 = nc.dram_tensor("gc1", [P, 3, 4, 2, P], F16, kind="ExternalInput")
    gc2 = nc.dram_tensor("gc2", [P, 2, 2, P], F16, kind="ExternalInput")
    gp3 = nc.dram_tensor("gp3", [P, 2, 2, 3, 4, P], F16, kind="ExternalInput")
    bm = nc.dram_tensor("bm", [P, 12], F32, kind="ExternalInput")
    # output as E/O plane pairs; the host interleaves + strips pads
    o = nc.dram_tensor("o", [2, P, 2, OH, C], F16, kind="ExternalOutput")

    groups = [[0, 1], [2, 3], [4, 5], [6, 7]]

    with TileContext(nc) as tc:
        with (
            tc.tile_pool(name="const", bufs=1) as cpool,
            tc.tile_pool(name="wt", bufs=1) as wpool,
            tc.tile_pool(name="psum", bufs=4, space="PSUM") as psum,
            tc.tile_pool(name="epi", bufs=2) as epool,
            tc.tile_pool(name="scratch", bufs=1) as spool_s,
            tc.tile_pool(name="dram", bufs=1, space="DRAM") as dpool,
            tc.tile_pool(name="s", bufs=1) as spool,
            tc.tile_pool(name="x", bufs=1) as xpool,
        ):
            bmt = cpool.tile([P, 12], F32)
            nc.sync.dma_start(bmt[:], bm[:])
            m_top = bmt[:, 6:7]
            m_bot = bmt[:, 7:8]
            coloff = bmt[:, 8:9]

            # ---- weights: gp2 first (prewarm + early p2 chunks) ----
            gp1t = wpool.tile([P, 2, 3, 4, P], F16, tag="wg1")
            gp2t = wpool.tile([P, 2, 3, 4, P], F16, tag="wg2")
            nc.sync.dma_start(gp2t[:, 0], gp2[:, 0])
            nc.sync.dma_start(gp2t[:, 1], gp2[:, 1])
            gc1t = wpool.tile([P, 3, 4, 2, P], F16, tag="wgc1")
            gc2t = wpool.tile([P, 2, 2, P], F16, tag="wgc2")

            # ---- PE pre-warm on gp2 ----
            gp2fl = gp2t[:].rearrange("p a b c d -> p (a b c d)")
            ptw = psum.tile([P, 4, NT], F32, tag="ps")
            for _ in range(3):
                nc.tensor.matmul(
                    ptw[:, 0, :], gp2fl[:, 0:P], gp2fl[:, 0:NT],
                    start=True, stop=True,
                )

            # ---- x t-planes + E/O planes ----
            xtt = [xpool.tile([P, 4, HS, C], F16, name=f"xt{s}", tag=f"xt{s}")
                   for s in range(2)]
            xet = [xpool.tile([P, 2, HS, C], F16, name=f"xe{s}", tag=f"xe{s}")
                   for s in range(2)]
            starts = [60, 0, 6, 54, 48, 42, 36, 30, 24, 18, 12]
            for k, r0 in enumerate(starts):
                for s in range(2):
                    nc.sync.dma_start(
                        xtt[s][:, :, r0:r0 + 6, :], xt[s, :, :, r0:r0 + 6, :]
                    )
                if k == 1:
                    nc.sync.dma_start(gp1t[:], gp1[:])
                elif k == 2:
                    nc.sync.dma_start(gc2t[:], gc2[:])
                elif k == 3:
                    nc.sync.dma_start(gc1t[:], gc1[:])
                elif k == 5:
                    nc.sync.dma_start(xet[0][:], xe[0])
                elif k == 7:
                    nc.sync.dma_start(xet[1][:], xe[1])
            xtf = [[xtt[s][:, w4].rearrange("p h w -> p (h w)") for w4 in range(4)]
                   for s in range(2)]
            xef = [[xet[s][:, eo].rearrange("p h w -> p (h w)") for eo in range(2)]
                   for s in range(2)]

            # ---- s planes (p1 output rows 2..65; halo rows 0,1,66,67) ----
            sE = spool.tile([P, SS, C], F16, name="sE", tag="sE")
            sO = spool.tile([P, SS, C], F16, name="sO", tag="sO")
            sEf = sE[:].rearrange("p h w -> p (h w)")
            sOf = sO[:].rearrange("p h w -> p (h w)")
            # rows 66,67 zeroed so the colmax trees can use them as
            # identity padding (p1 output is post-relu, >= 0)
            for sp_ in (sE, sO):
                nc.gpsimd.memset(sp_[:, SS - 2:SS, :], 0.0)

            tcs = spool_s.tile([P, NT], F16, tag="tcs")

            def epilogue(pt, nt, bias, dstE, dstO, act_heavy=False):
                """y0 = relu(m0+m1+m2+b) -> dstE, y1 = relu(m1-m2-m3+b) -> dstO.
                act_heavy also copies m2 to SBUF via ACT so the tensor adds
                run on GPSIMD, freeing DVE (used under p1 where DVE owns the
                cummax chain).  GPSIMD cannot touch PSUM, so the PSUM-reading
                ops always stay on DVE."""
                ta = epool.tile([P, NT], F32, tag="ta")
                tb = epool.tile([P, NT], F32, tag="tb")
                nc.scalar.activation(ta[:, :nt], pt[:, 1, :nt], AF.Copy)
                if act_heavy:
                    nc.scalar.activation(tcs[:, :nt], pt[:, 2, :nt], AF.Copy)
                    nc.vector.tensor_tensor(tb[:, :nt], ta[:, :nt],
                                            pt[:, 0, :nt], AL.add)
                    nc.gpsimd.tensor_tensor(tb[:, :nt], tb[:, :nt],
                                            tcs[:, :nt], AL.add)
                    nc.scalar.activation(dstE, tb[:, :nt], AF.Relu, bias=bias)
                    nc.gpsimd.tensor_tensor(ta[:, :nt], ta[:, :nt],
                                            tcs[:, :nt], AL.subtract)
                    nc.vector.tensor_tensor(ta[:, :nt], ta[:, :nt],
                                            pt[:, 3, :nt], AL.subtract)
                    nc.scalar.activation(dstO, ta[:, :nt], AF.Relu, bias=bias)
                    return
                nc.vector.tensor_tensor(tb[:, :nt], ta[:, :nt], pt[:, 0, :nt], AL.add)
                nc.vector.tensor_tensor(tb[:, :nt], tb[:, :nt], pt[:, 2, :nt], AL.add)
                nc.scalar.activation(dstE, tb[:, :nt], AF.Relu, bias=bias)
                nc.vector.tensor_tensor(ta[:, :nt], ta[:, :nt], pt[:, 2, :nt],
                                        AL.subtract)
                nc.vector.tensor_tensor(ta[:, :nt], ta[:, :nt], pt[:, 3, :nt],
                                        AL.subtract)
                nc.scalar.activation(dstO, ta[:, :nt], AF.Relu, bias=bias)

            def conv_p(gt, bias, outEf, outOf, out_base, chunk_list,
                       post_chunk=None, act_heavy=False):
                """p1/p2-style conv: K=256 (2 slabs), 128 out ch."""
                for i, (q, nt) in enumerate(chunk_list):
                    pt = psum.tile([P, 4, NT], F32, tag="ps")
                    for w4 in range(4):
                        terms = [(gt[:, s, ky, w4, :], xtf[s][w4], ky * C)
                                 for s in range(2) for ky in range(3)]
                        for j, (lhsT, rf, off) in enumerate(terms):
                            nc.tensor.matmul(
                                pt[:, w4, :nt], lhsT, rf[:, q + off:q + off + nt],
                                start=(j == 0), stop=(j == len(terms) - 1),
                            )
                    epilogue(pt, nt, bias,
                             outEf[:, out_base + q:out_base + q + nt],
                             outOf[:, out_base + q:out_base + q + nt],
                             act_heavy=act_heavy)
                    if post_chunk is not None:
                        post_chunk(q, nt)

            # ---- conv p2 exchange chunks (rows 62-63 and 0-6) first ----
            with tc.tile_pool(name="p2", bufs=1) as p2pool:
                # 66 rows (only 0..63 used) so the slot fits r-half0 later
                p2E = p2pool.tile([P, HS, C], F16, name="p2E", tag="p2E")
                p2O = p2pool.tile([P, HS, C], F16, name="p2O", tag="p2O")
                p2Ef = p2E[:].rearrange("p h w -> p (h w)")
                p2Of = p2O[:].rearrange("p h w -> p (h w)")
                mk = p2pool.tile([P, PCR, C], F16)
                nc.vector.memset(mk[:], 1.0)
                nc.vector.memset(mk[:, :, 0:1], 0.0)
                nc.vector.memset(mk[:, :, C - 1:C], 0.0)
                mkf = mk[:].rearrange("p h w -> p (h w)")

                # exchange-first chunk order: chunks 15,16 cover rows 62,63
                # and chunk 0 covers rows 0,1
                p2_fc = _chunks(FLAT_P, NT)
                p2_chunks = [p2_fc[15], p2_fc[16], p2_fc[0]] + p2_fc[1:15]
                conv_p(gp2t, bmt[:, 1:2], p2Ef, p2Of, 0, p2_chunks[:3])

                def wscan(rows_ap_E, rows_ap_O, mask_f):
                    # in-place W reverse cummax on an E/O row range
                    nc.vector.tensor_tensor(
                        rows_ap_O[:, :, 1:65], rows_ap_O[:, :, 1:65],
                        rows_ap_E[:, :, 2:66], AL.max,
                    )
                    flatO = rows_ap_O.rearrange("p h w -> p (h w)")
                    nc.vector.tensor_tensor_scan(
                        flatO[:, ::-1], flatO[:, ::-1], mask_f[:, ::-1],
                        0.0, AL.max, AL.mult,
                    )
                    nc.vector.tensor_tensor(
                        rows_ap_E[:], rows_ap_E[:], rows_ap_O[:], AL.max,
                    )

                for r in (62, 63, 0, 1):
                    nc.vector.memset(p2E[:, r:r + 1, C - 1:C], 0.0)
                    wscan(p2E[:, r:r + 1, :], p2O[:, r:r + 1, :],
                          mkf[:, 0:C])

                # ---- conv p1 (reverse order kept for DMA piece priority) ----
                p1_chunks = _chunks(FLAT_P, NT)
                conv_p(gp1t, bmt[:, 0:1], sEf, sOf, 2 * C,
                       list(reversed(p1_chunks)))

                # ---- colmax trees: the exchange payload rows are max-
                # reductions over the RAW p1 rows (max-reduce commutes with
                # the cummax chain), so the collective launches right after
                # conv p1 instead of after the 126-step chain.  Tree temps
                # live in p2 plane rows 21:53, which conv p2's later chunks
                # overwrite only after (WAR-ordered) the ct reads.
                def colmax_tree(sp, tdst, lo):
                    # tdst[0] = col-max of sp rows lo..lo+63 (rows 66,67 = 0)
                    nc.vector.tensor_tensor(
                        tdst[:, 0:32, :], sp[:, lo:lo + 32, :],
                        sp[:, lo + 32:lo + 64, :], AL.max)
                    n = 16
                    while n >= 1:
                        nc.vector.tensor_tensor(
                            tdst[:, 0:n, :], tdst[:, 0:n, :],
                            tdst[:, n:2 * n, :], AL.max)
                        n //= 2

                tE = p2E[:, 21:53, :]
                tO = p2O[:, 21:53, :]
                tE1 = p2E[:, 53:58, :]
                tO1 = p2O[:, 53:58, :]

                # ---- pairwise exchange (E|O concat, 132-wide rows).
                # ct build + u run on GPSIMD so they fire on data-readiness
                # instead of queueing behind the DVE backlog.
                ct = spool_s.tile([P, 8, 2 * C], F16, tag="exch")

                def ct_slot(k, se_, so_, m):
                    nc.gpsimd.tensor_scalar_mul(ct[:, k, 0:C], se_, m)
                    nc.gpsimd.tensor_scalar_mul(ct[:, k, C:2 * C], so_, m)

                colmax_tree(sE, tE, 2)       # tE[0] = cp1local row 2
                colmax_tree(sO, tO, 2)
                ct_slot(0, tE[:, 0, :], tO[:, 0, :], m_bot)
                colmax_tree(sE, tE, 3)       # tE[0] = cp1local row 3
                colmax_tree(sO, tO, 3)
                ct_slot(1, tE[:, 0, :], tO[:, 0, :], m_bot)
                # cp1local rows 64,65: max(raw64, raw65) and raw65 itself
                nc.vector.tensor_tensor(
                    tE1[:, 0, :], sE[:, 2 + 62, :], sE[:, 2 + 63, :], AL.max)
                nc.vector.tensor_tensor(
                    tO1[:, 0, :], sO[:, 2 + 62, :], sO[:, 2 + 63, :], AL.max)
                ct_slot(2, tE1[:, 0, :], tO1[:, 0, :], m_top)
                ct_slot(3, sE[:, 2 + OH - 1, :], sO[:, 2 + OH - 1, :], m_top)
                ct_slot(4, p2E[:, 0, :], p2O[:, 0, :], m_bot)
                ct_slot(5, p2E[:, 1, :], p2O[:, 1, :], m_bot)
                ct_slot(6, p2E[:, OH - 2, :], p2O[:, OH - 2, :], m_top)
                ct_slot(7, p2E[:, OH - 1, :], p2O[:, OH - 1, :], m_top)
                cc_in = dpool.tile([P, 8, 2 * C], F16)
                cc_out = dpool.tile([P, 8, 2 * C], F16)
                nc.sync.dma_start(cc_in[:], ct[:])
                nc.gpsimd.collective_compute(
                    "AllReduce", AL.add, replica_groups=groups,
                    ins=[cc_in[:]], outs=[cc_out[:]],
                )
                rx = spool_s.tile([P, 8, 2 * C], F16, tag="exch2")
                nc.sync.dma_start(rx[:], cc_out[:])

                # u = R[0] + coloff (top: partner colmax; bottom: -inf)
                u = spool_s.tile([P, 2 * C], F16, tag="u")
                nc.gpsimd.tensor_scalar_add(u[:], rx[:, 0, :], coloff)

                # ---- s-plane t-transform target ----
                with tc.tile_pool(name="st", bufs=1) as stpool:
                    st = stpool.tile([P, 4, SS, C], F16)
                    nc.vector.memset(st[:, :, :, 0:1], 0.0)
                    nc.vector.memset(st[:, :, :, C - 1:C], 0.0)
                    stf = [st[:, w4].rearrange("p h w -> p (h w)")
                           for w4 in range(4)]
                    cm = spool_s.tile([P, 2 * C], F16, tag="cm")
                    h0 = spool_s.tile([P, C], F16, tag="h0")
                    h1 = spool_s.tile([P, C], F16, tag="h1")

                    def st_xform(r0, nr):
                        # st rows r0..r0+nr from s rows (same tile rows)
                        args = [
                            (st[:, 0, r0:r0 + nr, 1:65],
                             sO[:, r0:r0 + nr, 0:64], sO[:, r0:r0 + nr, 1:65],
                             AL.subtract),
                            (st[:, 1, r0:r0 + nr, 1:65],
                             sE[:, r0:r0 + nr, 1:65], sO[:, r0:r0 + nr, 1:65],
                             AL.add),
                            (st[:, 2, r0:r0 + nr, 1:65],
                             sO[:, r0:r0 + nr, 1:65], sE[:, r0:r0 + nr, 1:65],
                             AL.subtract),
                            (st[:, 3, r0:r0 + nr, 1:65],
                             sE[:, r0:r0 + nr, 1:65], sE[:, r0:r0 + nr, 2:66],
                             AL.subtract),
                        ]
                        for i, (d, a, b_, op) in enumerate(args):
                            eng = nc.vector if i % 2 == 0 else nc.gpsimd
                            eng.tensor_tensor(d, a, b_, op)

                    def piece_scan(pc):
                        # u-independent part: W reverse cummax of the piece
                        # (max ops must stay on DVE; Pool has no max)
                        r0 = pc * PCR
                        nc.vector.memset(p2E[:, r0:r0 + PCR, C - 1:C], 0.0)
                        wscan(p2E[:, r0:r0 + PCR, :], p2O[:, r0:r0 + PCR, :],
                              mkf)

                    def piece_fix(pc):
                        # collective-dependent part: colmax fixup, s=cp1+cp2,
                        # pads, t-transform
                        r0 = pc * PCR
                        sr0 = 2 + r0
                        for sp_, uc in ((sE, 0), (sO, C)):
                            nc.vector.tensor_tensor(
                                sp_[:, sr0:sr0 + PCR, :],
                                sp_[:, sr0:sr0 + PCR, :],
                                u[:, None, uc:uc + C].to_broadcast((P, PCR, C)),
                                AL.max,
                            )
                        if pc == 0:
                            nc.vector.tensor_copy(cm[:, 0:C], sE[:, 2, :])
                            nc.vector.tensor_copy(cm[:, C:2 * C], sO[:, 2, :])
                        nc.gpsimd.tensor_tensor(
                            sE[:, sr0:sr0 + PCR, :], sE[:, sr0:sr0 + PCR, :],
                            p2E[:, r0:r0 + PCR, :], AL.add)
                        nc.gpsimd.tensor_tensor(
                            sO[:, sr0:sr0 + PCR, :], sO[:, sr0:sr0 + PCR, :],
                            p2O[:, r0:r0 + PCR, :], AL.add)
                        for sp_ in (sE, sO):
                            nc.gpsimd.memset(sp_[:, sr0:sr0 + PCR, 0:1], 0.0)
                            nc.gpsimd.memset(sp_[:, sr0:sr0 + PCR, C - 1:C], 0.0)
                        if pc == 0:
                            # halo rows: above (bottom cores) s rows 0,1
                            for j in range(2):
                                for sp_, cc0 in ((sE, 0), (sO, C)):
                                    nc.vector.tensor_tensor(
                                        h0[:], rx[:, 2 + j, cc0:cc0 + C],
                                        cm[:, cc0:cc0 + C], AL.max)
                                    nc.vector.tensor_tensor(
                                        h0[:], h0[:], rx[:, 6 + j, cc0:cc0 + C],
                                        AL.add)
                                    nc.vector.tensor_scalar_mul(
                                        sp_[:, j, :], h0[:], m_bot)
                            # below (top cores) s rows 66,67
                            for j in range(2):
                                for sp_, cc0 in ((sE, 0), (sO, C)):
                                    nc.vector.tensor_tensor(
                                        h1[:], rx[:, 0 + j, cc0:cc0 + C],
                                        rx[:, 4 + j, cc0:cc0 + C], AL.add)
                                    nc.vector.tensor_scalar_mul(
                                        sp_[:, SS - 2 + j, :], h1[:], m_top)
                            for sp_ in (sE, sO):
                                for rr in (0, SS - 2):
                                    nc.vector.memset(
                                        sp_[:, rr:rr + 2, 0:1], 0.0)
                                    nc.vector.memset(
                                        sp_[:, rr:rr + 2, C - 1:C], 0.0)
                            st_xform(0, 2)
                            st_xform(SS - 2, 2)
                        st_xform(sr0, PCR)

                    # ---- conv p2 remaining chunks.  The cummax chain (126
                    # DVE steps) and the u-independent W-scans drain under
                    # p2's matmuls; the u-dependent fixups run after, right
                    # as the collective result lands. ----
                    next_pc = [0]
                    chain_h = [OH]

                    def p2_post(q, nt):
                        hi = chain_h[0]
                        for h in range(hi, max(hi - 20, 1), -1):
                            nc.vector.tensor_tensor(
                                sE[:, h, :], sE[:, h, :], sE[:, h + 1, :],
                                AL.max)
                            nc.vector.tensor_tensor(
                                sO[:, h, :], sO[:, h, :], sO[:, h + 1, :],
                                AL.max)
                        chain_h[0] = max(hi - 20, 1)
                        covered = (q + nt) // C
                        while (next_pc[0] < NPC
                               and covered >= PCR * (next_pc[0] + 1)):
                            piece_scan(next_pc[0])
                            next_pc[0] += 1

                    conv_p(gp2t, bmt[:, 1:2], p2Ef, p2Of, 0, p2_chunks[3:],
                           post_chunk=p2_post)
                    for h in range(chain_h[0], 1, -1):
                        nc.vector.tensor_tensor(
                            sE[:, h, :], sE[:, h, :], sE[:, h + 1, :], AL.max)
                        nc.vector.tensor_tensor(
                            sO[:, h, :], sO[:, h, :], sO[:, h + 1, :], AL.max)
                    while next_pc[0] < NPC:
                        piece_scan(next_pc[0])
                        next_pc[0] += 1
                    for pc in range(NPC):
                        piece_fix(pc)

                    # gp3 into the slots gp1/gp2 free after their last chunks
                    # (emitted only now that every gp1/gp2 reader exists)
                    gp3t = [wpool.tile([P, 2, 3, 4, P], F16, name=f"gp3{t}",
                                       tag=t) for t in ("wg1", "wg2")]
                    nc.sync.dma_start(gp3t[0][:], gp3[:, 0])
                    nc.sync.dma_start(gp3t[1][:], gp3[:, 1])

                    # ---- r t-plane targets (xt slots; pads cleared) ----
                    rtt = []
                    for half in range(2):
                        rt_ = xpool.tile([P, 4, HS, C], F16, tag=f"xt{half}")
                        nc.vector.memset(rt_[:, :, :, 0:1], 0.0)
                        nc.vector.memset(rt_[:, :, :, C - 1:C], 0.0)
                        rtt.append(rt_)
                    rtf = [[rtt[s][:, w4].rearrange("p h w -> p (h w)")
                            for w4 in range(4)] for s in range(2)]

                    # ---- conv c1 (+ folded c2) -> r planes ----
                    c1_chunks = _chunks(FLAT_C, NT)
                    for half in range(2):
                        if half == 0:
                            rE = p2pool.tile([P, HS, C], F16, tag="p2E")
                            rO = p2pool.tile([P, HS, C], F16, tag="p2O")
                        else:
                            rE = spool.tile([P, HS, C], F16, tag="sE")
                            rO = spool.tile([P, HS, C], F16, tag="sO")
                        rEf = rE[:].rearrange("p h w -> p (h w)")
                        rOf = rO[:].rearrange("p h w -> p (h w)")
                        for i, (q, nt) in enumerate(c1_chunks):
                            pt = psum.tile([P, 4, NT], F32, tag="ps")
                            for w4 in range(4):
                                terms = [(gc1t[:, ky, w4, half, :], stf[w4],
                                          ky * C) for ky in range(3)]
                                if w4 == 0:
                                    terms += [(gc2t[:, s, half, :], xef[s][0], 0)
                                              for s in range(2)]
                                elif w4 == 3:
                                    terms += [(gc2t[:, s, half, :], xef[s][1], 0)
                                              for s in range(2)]
                                for j, (lhsT, rf, off) in enumerate(terms):
                                    nc.tensor.matmul(
                                        pt[:, w4, :nt], lhsT,
                                        rf[:, q + off:q + off + nt],
                                        start=(j == 0),
                                        stop=(j == len(terms) - 1),
                                    )
                            epilogue(pt, nt, bmt[:, 2 + half:3 + half],
                                     rEf[:, q:q + nt], rOf[:, q:q + nt],
                                     act_heavy=False)
                        # mask invalid halo rows, zero pads, transform to
                        # rt right away (overlaps the other half's matmuls)
                        for rp_ in (rE, rO):
                            nc.vector.tensor_scalar_mul(
                                rp_[:, 0, :], rp_[:, 0, :], m_bot)
                            nc.vector.tensor_scalar_mul(
                                rp_[:, HS - 1, :], rp_[:, HS - 1, :], m_top)
                            nc.vector.memset(rp_[:, :, 0:1], 0.0)
                            nc.vector.memset(rp_[:, :, C - 1:C], 0.0)
                        rt_ = rtt[half]
                        for r0, nr in ((0, 17), (17, 17), (34, 16), (50, 16)):
                            args = [
                                (rt_[:, 0, r0:r0 + nr, 1:65],
                                 rO[:, r0:r0 + nr, 0:64],
                                 rO[:, r0:r0 + nr, 1:65], AL.subtract),
                                (rt_[:, 1, r0:r0 + nr, 1:65],
                                 rE[:, r0:r0 + nr, 1:65],
                                 rO[:, r0:r0 + nr, 1:65], AL.add),
                                (rt_[:, 2, r0:r0 + nr, 1:65],
                                 rO[:, r0:r0 + nr, 1:65],
                                 rE[:, r0:r0 + nr, 1:65], AL.subtract),
                                (rt_[:, 3, r0:r0 + nr, 1:65],
                                 rE[:, r0:r0 + nr, 1:65],
                                 rE[:, r0:r0 + nr, 2:66], AL.subtract),
                            ]
                            for i, (d, a, b_, op) in enumerate(args):
                                eng = nc.vector if i % 2 == 0 else nc.gpsimd
                                eng.tensor_tensor(d, a, b_, op)

                    # ---- conv p3 -> E/O output planes (host interleaves) ----
                    p3_chunks = _chunks(FLAT_P, NT)
                    for half in range(2):
                        oeo = xpool.tile([P, 2, OH, C], F16, tag=f"xe{half}")
                        oEf = oeo[:, 0].rearrange("p h w -> p (h w)")
                        oOf = oeo[:, 1].rearrange("p h w -> p (h w)")
                        r_sent = [0]

                        def p3_post(q, nt, half=half, oeo=oeo, r_sent=r_sent):
                            # ship completed rows as soon as relu lands
                            r_done = (q + nt) // C
                            if r_done - r_sent[0] >= 16 or q + nt == FLAT_P:
                                if q + nt == FLAT_P:
                                    r_done = OH
                                nc.sync.dma_start(
                                    o[half, :, :, r_sent[0]:r_done, :],
                                    oeo[:, :, r_sent[0]:r_done, :])
                                r_sent[0] = r_done

                        for i, (q, nt) in enumerate(p3_chunks):
                            pt = psum.tile([P, 4, NT], F32, tag="ps")
                            for w4 in range(4):
                                terms = [(gp3t[s][:, half, ky, w4, :],
                                          rtf[s][w4], ky * C)
                                         for s in range(2) for ky in range(3)]
                                for j, (lhsT, rf, off) in enumerate(terms):
                                    nc.tensor.matmul(
                                        pt[:, w4, :nt], lhsT,
                                        rf[:, q + off:q + off + nt],
                                        start=(j == 0),
                                        stop=(j == len(terms) - 1),
                                    )
                            epilogue(pt, nt, bmt[:, 4 + half:5 + half],
                                     oEf[:, q:q + nt], oOf[:, q:q + nt])
                            p3_post(q, nt)

    _fix_multiwaits(nc)
    return nc


_NC = None


def _get_nc():
    global _NC
    if _NC is None:
        _NC = build_nc()
    return _NC


def _fold_bn(w, g, b, m, v):
    s = (np.asarray(g) / np.sqrt(np.asarray(v) + EPS)).astype(np.float32)
    t = (np.asarray(b) - np.asarray(m) * s).astype(np.float32)
    return np.asarray(w, np.float32) * s[:, None, None, None], t


def _wino_w(w):
    # w [O, I, 3, 3] -> G [4, 3ky, I, O]
    g0, g1, g2 = w[..., 0], w[..., 1], w[..., 2]
    G = np.stack([g0, (g0 + g1 + g2) * 0.5, (g0 - g1 + g2) * 0.5, g2])
    return G.transpose(0, 3, 2, 1).astype(np.float16)


def kernel(**inputs):
    x = np.asarray(inputs["x"], np.float32)

    w_p1, t_p1 = _fold_bn(inputs["w_p1"], inputs["g_p1"], inputs["b_p1"],
                          inputs["m_p1"], inputs["v_p1"])
    w_p2, t_p2 = _fold_bn(inputs["w_p2"], inputs["g_p2"], inputs["b_p2"],
                          inputs["m_p2"], inputs["v_p2"])
    w_c1, t_c1 = _fold_bn(inputs["w_c1"], inputs["g_c1"], inputs["b_c1"],
                          inputs["m_c1"], inputs["v_c1"])
    w_c2, t_c2 = _fold_bn(inputs["w_c2"], inputs["g_c2"], inputs["b_c2"],
                          inputs["m_c2"], inputs["v_c2"])
    w_p3, t_p3 = _fold_bn(inputs["w_p3"], inputs["g_p3"], inputs["b_p3"],
                          inputs["m_p3"], inputs["v_p3"])

    Gp1 = _wino_w(w_p1)  # [4,3,256,128]
    Gp2 = _wino_w(w_p2)
    Gc1 = _wino_w(w_c1)  # [4,3,128,256]
    Gp3 = _wino_w(w_p3)  # [4,3,256,256]

    gp1a = np.ascontiguousarray(
        Gp1.reshape(4, 3, 2, P, P).transpose(3, 2, 1, 0, 4))
    gp2a = np.ascontiguousarray(
        Gp2.reshape(4, 3, 2, P, P).transpose(3, 2, 1, 0, 4))
    gc1a = np.ascontiguousarray(
        Gc1.reshape(4, 3, P, 2, P).transpose(2, 1, 0, 3, 4))
    gp3a = np.ascontiguousarray(
        Gp3.reshape(4, 3, 2, P, 2, P).transpose(3, 2, 4, 1, 0, 5))
    gc2a = np.ascontiguousarray(
        w_c2[:, :, 0, 0].reshape(2, P, 2, P).transpose(3, 2, 0, 1)
    ).astype(np.float16)

    bias = np.zeros((P, 6), np.float32)
    bias[:, 0] = t_p1
    bias[:, 1] = t_p2
    bc = t_c1 + t_c2
    bias[:, 2] = bc[:P]
    bias[:, 3] = bc[P:]
    bias[:, 4] = t_p3[:P]
    bias[:, 5] = t_p3[P:]

    # x slabs per core-half with H halo, as fp16 E/O planes + t-planes
    x16 = x.astype(np.float16).astype(np.float32)
    xr = x16.reshape(B, 2, P, H, W)
    pad = np.zeros((B, 2, 2, P, HS, W), np.float32)  # [b, half, slab, p, h, w]
    pad[:, 0, :, :, 1:HS, :] = xr[:, :, :, 0:65, :]
    pad[:, 1, :, :, 0:HS - 1, :] = xr[:, :, :, 63:128, :]
    xE = np.zeros((B, 2, 2, P, HS, C), np.float32)
    xO = np.zeros_like(xE)
    xE[..., 1:65] = pad[..., 0::2]
    xO[..., 1:65] = pad[..., 1::2]
    t4 = np.zeros((B, 2, 2, P, 4, HS, C), np.float32)
    t4[..., 0, :, 1:65] = xO[..., 0:64] - xO[..., 1:65]
    t4[..., 1, :, 1:65] = xE[..., 1:65] + xO[..., 1:65]
    t4[..., 2, :, 1:65] = xO[..., 1:65] - xE[..., 1:65]
    t4[..., 3, :, 1:65] = xE[..., 1:65] - xE[..., 2:66]
    t4 = t4.astype(np.float16)
    xeo = np.stack([xE, -xO], axis=4).astype(np.float16)  # [b,half,slab,p,2,h,c]

    wmaps = {"gp1": gp1a, "gp2": gp2a, "gc1": gc1a, "gc2": gc2a, "gp3": gp3a}
    in_maps = []
    for b in range(B):
        for half in range(2):
            bmv = np.zeros((P, 12), np.float32)
            bmv[:, 0:6] = bias
            if half == 0:  # top
                bmv[:, 6] = 1.0
                bmv[:, 8] = 0.0
            else:  # bottom
                bmv[:, 7] = 1.0
                bmv[:, 8] = -1e30
            in_maps.append({
                "xt": t4[b, half], "xe": xeo[b, half], "bm": bmv, **wmaps,
            })

    global _last_in_maps
    _last_in_maps = in_maps

    nc = _get_nc()
    res = run_bass_kernel_spmd(nc, in_maps, list(range(8)))

    out = np.empty((B, CIN, H, W), np.float32)
    for b in range(B):
        for half in range(2):
            a = np.asarray(res.results[2 * b + half]["o"], np.float32)
            v = a[:, :, :, :, 1:65]  # [2ch, 128, 2eo, 64, 64]
            rows = slice(half * OH, (half + 1) * OH)
            out[b, :, rows, 0::2] = v[:, :, 0].reshape(CIN, OH, 64)
            out[b, :, rows, 1::2] = v[:, :, 1].reshape(CIN, OH, 64)
    return out


if __name__ == "__main__":
    import reference

    inp = {k: np.asarray(v) for k, v in reference.setup_inputs().items()}
    exp = np.asarray(reference.reference(**inp))
    got = kernel(**inp)
    err = np.abs(got - exp)
    rel = err.max() / max(np.abs(exp).max(), 1e-6)
    print("abs err max:", err.max(), "rel (vs absmax):", rel)
